# revision 38
# baseline (speedup 1.0000x reference)
"""Trainium2 8-core kernel for nn_Attention_27530740367526.

Multi-head causal attention (B=2, S=2048, D=2048, H=16, HD=128) with RoPE,
sharded batch x head-group across 8 NeuronCores: core c handles batch c//4
and heads [4*(c%4), 4*(c%4)+4).  Each core computes q/k/v projections
(+RoPE), attention for its 4 heads, and its heads' slice of the wo
projection -- a partial [S, D] output.  The host sums the 4 partials per
batch (the row-parallel wo "all-reduce" is a host-side unshard).

All matmul operands are bf16 (PSUM accumulation is fp32), which runs at
full PE rate, halves DMA/SBUF traffic vs f32r, and keeps LDWEIGHTS cheap.
Everything lives in "transposed land": qT/kT are [head_dim, seq] with
head-dim on partitions, so scores come out transposed ([k, q]), the
softmax denominator is an all-ones-column matmul (partition-broadcast
denominator for free), and PV / wo consume natural layouts with zero
on-device transposes.  RoPE's rotate-half is a 128x128 permutation matmul.

Schedule per core (single pass over all 4 heads -- y is written once):
  P0 A0 P1 A1+W0 P2 A2+W1 P3 A3+W2 W3
where P(sc) projects q/k/v for 512-seq chunk sc (dense PE phase, next x
chunk prefetched via split DMA queues), A(qc) runs causal attention for
query chunk qc as two 2-head interleaved softmax chains, and W(qc) is the
wo projection of chunk qc cut into 16 [128,512] blocks used as PE filler
inside the NEXT attention phase's exp-wait bubbles (one 4-matmul block
between a step's exp and its PV keeps the PE continuously busy, which
also keeps the PE p-state clock at max).
"""

import sys

if "/opt/trn_rl_repo" not in sys.path:
    sys.path.insert(0, "/opt/trn_rl_repo")

from collections import deque

import ml_dtypes
import numpy as np

import concourse.bacc as bacc
import concourse.mybir as mybir
import concourse.tile as tile
from concourse.bass_utils import run_bass_kernel_spmd

F32 = mybir.dt.float32
BF16 = mybir.dt.bfloat16
AF = mybir.ActivationFunctionType
BF_NP = ml_dtypes.bfloat16

N_HEADS = 16
N_CORES = 8
B, S, D = 2, 2048, 2048
HD = D // N_HEADS
H_LOC = N_HEADS // (N_CORES // B)  # 4 heads per core
HW = H_LOC * HD                    # 512 wo rows per core
SC = 512                           # seq chunk (matmul moving free dim)
P = 128
KO = D // P                        # 16 contraction subtiles
NQC = S // SC                      # 4 q-chunks
NSUB = SC // P                     # 4 128-blocks per chunk
NST = S // P                       # 16 s-tiles
LA = 2                             # scores-tile software pipeline depth


def _build_core_kernel(causal: bool):
    inv_sqrt_hd = 1.0 / float(np.sqrt(HD))

    nc = bacc.Bacc(None, target_bir_lowering=False)

    # All inputs are pre-swizzled on the host so every DMA descriptor is
    # one segment per partition row (contiguous 1-16 KB rows): fat issues
    # were measured at 3-12 us on the issuing engine otherwise.
    xT = nc.dram_tensor("xT", [D, S], BF16, kind="ExternalInput")
    wqkP = nc.dram_tensor("wqkP", [P, 8, KO, HD], BF16, kind="ExternalInput")
    wvP = nc.dram_tensor("wvP", [P, KO, 4 * HD], BF16, kind="ExternalInput")
    woP = nc.dram_tensor("woP", [P, H_LOC, D], BF16, kind="ExternalInput")
    cosT = nc.dram_tensor("cosT", [HD, S], BF16, kind="ExternalInput")
    sinT = nc.dram_tensor("sinT", [HD, S], BF16, kind="ExternalInput")
    PT = nc.dram_tensor("PT", [HD, HD], BF16, kind="ExternalInput")
    ones = nc.dram_tensor("ones", [P, P], BF16, kind="ExternalInput")
    if causal:
        # bf16 is plenty: mask entries are 0 or ~-1e10, and exp of any
        # value <= -1e8 is 0 either way
        maskP = nc.dram_tensor("maskP", [P, NSUB, SC], BF16, kind="ExternalInput")
    else:
        maskT = nc.dram_tensor("maskT", [S, S], F32, kind="ExternalInput")
    y = nc.dram_tensor("y", [S, D], BF16, kind="ExternalOutput")

    xT_r = xT.rearrange("(ko ki) s -> ki ko s", ki=P)

    with tile.TileContext(nc) as tc:
        with (
            tc.tile_pool(name="persist", bufs=1) as persist,
            tc.tile_pool(name="xa", bufs=2) as xa,
            tc.tile_pool(name="qp", bufs=2) as qpool,
            tc.tile_pool(name="op", bufs=2) as opool,
            tc.tile_pool(name="plainp", bufs=3) as plainp,
            tc.tile_pool(name="ropet", bufs=2) as ropet,
            tc.tile_pool(name="ep", bufs=5) as ep,
            tc.tile_pool(name="yo", bufs=3) as yop,
            tc.tile_pool(name="scr", bufs=2) as scrp,
            tc.tile_pool(name="gm", bufs=3) as gmp,
            tc.tile_pool(name="acc", bufs=4, space="PSUM") as accp,
            tc.tile_pool(name="sc2", bufs=LA, space="PSUM") as sc2,
            tc.tile_pool(name="y2", bufs=2, space="PSUM") as y2,
        ):
            # ---- initial DMAs.  All weight blocks go on the scalar queue
            # (cheap single-segment issues; the scalar engine runs nothing
            # else early since PSUM->SBUF copies live on DVE); x chunk 0
            # round-robins sync/gpsimd per ko so the first chains can sweep
            # as subtiles land; cos/sin/mask/wo follow behind.
            wqk_sb = persist.tile([P, 8, KO, HD], BF16, tag="w", name="wqk_sb")
            wv_sb = persist.tile([P, KO, 4 * HD], BF16, tag="wv", name="wv_sb")
            xt0 = xa.tile([P, KO, SC], BF16, tag="xt", name="xt0")

            KH = KO // 2

            def w_half(eng, i, hf):
                eng.dma_start(
                    wqk_sb[:, i, hf * KH : (hf + 1) * KH],
                    wqkP[:, i, hf * KH : (hf + 1) * KH],
                )

            def wv_quarter(eng, q):
                eng.dma_start(
                    wv_sb[:, q * 4 : (q + 1) * 4], wvP[:, q * 4 : (q + 1) * 4]
                )

            # scalar: chains 0-3 weights (+late v quarters); sync carries
            # x chunk 0 evens + small persists, then chains 4-7 weights
            for i in range(4):
                w_half(nc.scalar, i, 0)
                w_half(nc.scalar, i, 1)
            for ko in range(KO):
                eng = nc.sync if ko % 2 == 0 else nc.gpsimd
                eng.dma_start(xt0[:, ko], xT_r[:, ko, 0:SC])
            wv_quarter(nc.scalar, 2)
            wv_quarter(nc.scalar, 3)
            cos_sb = persist.tile([P, S], BF16, tag="cos", name="cos_sb")
            nc.sync.dma_start(cos_sb[:], cosT[:])
            sin_sb = persist.tile([P, S], BF16, tag="sin", name="sin_sb")
            nc.gpsimd.dma_start(sin_sb[:], sinT[:])
            pt_sb = persist.tile([P, HD], BF16, tag="pt", name="pt_sb")
            nc.sync.dma_start(pt_sb[:], PT[:])
            ones_sb = persist.tile([P, P], BF16, tag="ones", name="ones_sb")
            nc.sync.dma_start(ones_sb[:], ones[:])
            for i in range(4, 8):
                w_half(nc.gpsimd, i, 0)
                w_half(nc.gpsimd, i, 1)
            wv_quarter(nc.sync, 0)
            wv_quarter(nc.sync, 1)
            if causal:
                mask_sb = persist.tile([P, NSUB, SC], BF16, tag="mask", name="mask_sb")
                nc.scalar.dma_start(mask_sb[:], maskP[:])
            wo_sb = persist.tile([P, H_LOC, D], BF16, tag="wo", name="wo_sb")
            nc.gpsimd.dma_start(wo_sb[:], woP[:])

            kT_sb = persist.tile([P, H_LOC, S], BF16, tag="kT", name="kT_sb")
            v_sb = persist.tile([P, NST, H_LOC * HD], BF16, tag="v", name="v_sb")
            qT_full = (
                persist.tile([P, H_LOC, S], BF16, tag="qTf", name="qT_full")
                if not causal
                else None
            )

            def load_chunk(sc):
                # prefetched a full phase ahead -> two half-descriptors
                ssl = slice(sc * SC, (sc + 1) * SC)
                xt = xa.tile([P, KO, SC], BF16, tag="xt", name=f"xt{sc}")
                nc.sync.dma_start(xt[:, : KO // 2], xT_r[:, : KO // 2, ssl])
                nc.gpsimd.dma_start(xt[:, KO // 2 :], xT_r[:, KO // 2 :, ssl])
                return xt

            def project_chunk(sc, xt, qT_c, do_v=True):
                """q/k (+RoPE) and v projections for seq chunk sc.  The
                RoPE for chain i is emitted during chain i+1's matmuls so
                the rotate-half matmul never stalls the PE on the
                PSUM->SBUF copy."""
                ssl = slice(sc * SC, (sc + 1) * SC)
                pending_rope = []

                def flush_rope():
                    for h, t, plain, dst in pending_rope:
                        rot = sc2.tile([P, SC], F32, tag="sc", name="rot")
                        nc.tensor.matmul(rot[:], pt_sb[:], plain[:])
                        pc = ropet.tile([P, SC], F32, tag="pc", name="pc")
                        nc.vector.tensor_mul(pc[:], plain[:], cos_sb[:, ssl])
                        t2 = ropet.tile([P, SC], F32, tag="t2", name="t2")
                        nc.vector.tensor_mul(t2[:], rot[:], sin_sb[:, ssl])
                        nc.vector.tensor_add(dst, pc[:], t2[:])
                    pending_rope.clear()

                for h in range(H_LOC):
                    for t in range(2):  # 0=q, 1=k
                        ps = accp.tile([P, SC], F32, tag="acc", name="ps")
                        for ko in range(KO):
                            nc.tensor.matmul(
                                ps[:],
                                wqk_sb[:, 2 * h + t, ko],
                                xt[:, ko],
                                start=(ko == 0),
                                stop=(ko == KO - 1),
                            )
                        plain = plainp.tile([P, SC], BF16, tag="plain", name="plain")
                        nc.vector.tensor_copy(plain[:], ps[:])
                        if t == 0:
                            dst = qT_c[:, h, ssl] if qT_c is qT_full else qT_c[:, h, :]
                        else:
                            dst = kT_sb[:, h, ssl]
                        flush_rope()
                        pending_rope.append((h, t, plain, dst))

                for sti in range(NSUB):
                    if do_v:
                        v_chain(sc, xt, sti, accp)
                    flush_rope()
                if not do_v:
                    flush_rope()

            def v_chain(sc, xt, sti, pool):
                st = sc * NSUB + sti
                lsl = slice(sti * P, (sti + 1) * P)
                psv = pool.tile(
                    [P, H_LOC * HD], F32,
                    tag="acc" if pool is accp else "y", name="psv",
                )
                for ko in range(KO):
                    nc.tensor.matmul(
                        psv[:],
                        xt[:, ko, lsl],
                        wv_sb[:, ko],
                        start=(ko == 0),
                        stop=(ko == KO - 1),
                    )
                nc.vector.tensor_copy(v_sb[:, st, :], psv[:])

            def attend_half(qc, half, qT_c, outT_qc, fillers):
                """Attention for query chunk qc, heads (2*half, 2*half+1)
                interleaved per k-block.  One filler block (4 wo matmuls)
                is drained between a step's exp and its PV matmul so the
                PE bridges the exp latency with independent work.

                Diagonal k-blocks (j = kb - qc*NSUB >= 0) are column-
                trimmed: only q columns >= j*P can attend to that block,
                so scores/exp/PV/denominator run on [:, j*P:] and the mask
                add touches just the [128,128] triangle."""
                nkb = (qc + 1) * NSUB if causal else NST
                hs = (2 * half, 2 * half + 1)
                qt = {}
                o_ps = {}
                d_ps = {}
                for hp in range(2):
                    qt[hp] = (
                        qT_c[:, hs[hp], qc * SC : (qc + 1) * SC]
                        if qT_c is qT_full
                        else qT_c[:, hs[hp], :]
                    )
                    o_ps[hp] = accp.tile([P, SC], F32, tag="acc", name=f"o{hp}")
                    d_ps[hp] = accp.tile([P, SC], F32, tag="acc", name=f"d{hp}")
                stile = {}

                def cotrim(kb):
                    j = kb - qc * NSUB
                    return P * j if (causal and j > 0) else 0

                # qc 0 has no wo fillers; deepen its scores lookahead by
                # borrowing the (idle until A(1)) y2 PSUM slots
                la = 4 if (causal and qc == 0) else LA
                scnt = [0]

                def emit_scores(kb, hp):
                    co = cotrim(kb)
                    if la == 4 and scnt[0] % 2 == 1:
                        t_ = y2.tile([P, SC], F32, tag="y", name="sc_y")
                    else:
                        t_ = sc2.tile([P, SC], F32, tag="sc", name="scores")
                    scnt[0] += 1
                    nc.tensor.matmul(
                        t_[:, co:],
                        kT_sb[:, hs[hp], kb * P : (kb + 1) * P],
                        qt[hp][:, co:],
                        skip_group_check=True,
                    )
                    if causal:
                        j = kb - qc * NSUB
                        if j >= 0:
                            nc.vector.tensor_add(
                                t_[:, co : co + P],
                                t_[:, co : co + P],
                                mask_sb[:, j, co : co + P],
                            )
                    else:
                        if hp == 0:
                            mt = gmp.tile([P, SC], F32, tag="mt", name="mt")
                            nc.sync.dma_start(
                                mt[:],
                                maskT[
                                    kb * P : (kb + 1) * P,
                                    qc * SC : (qc + 1) * SC,
                                ],
                            )
                            stile[("m", kb)] = mt
                        nc.vector.tensor_add(t_[:], t_[:], stile[("m", kb)][:])
                    stile[(kb, hp)] = t_

                seq = [(kb, hp) for kb in range(nkb) for hp in range(2)]
                for s_ in seq[:la]:
                    emit_scores(*s_)
                for i, (kb, hp) in enumerate(seq):
                    co = cotrim(kb)
                    e = ep.tile([P, SC], BF16, tag="e", name="e")
                    nc.scalar.activation(
                        e[:, co:],
                        stile.pop((kb, hp))[:, co:],
                        AF.Exp,
                        scale=inv_sqrt_hd,
                    )
                    if fillers and i % 2 == 0:
                        fillers.popleft()()
                    h = hs[hp]
                    nc.tensor.matmul(
                        o_ps[hp][:, co:],
                        v_sb[:, kb, h * HD : (h + 1) * HD],
                        e[:, co:],
                        start=(kb == 0),
                        stop=(kb == nkb - 1),
                        skip_group_check=True,
                    )
                    nc.tensor.matmul(
                        d_ps[hp][:, co:],
                        ones_sb[:],
                        e[:, co:],
                        start=(kb == 0),
                        stop=(kb == nkb - 1),
                        skip_group_check=True,
                    )
                    if i + la < len(seq):
                        emit_scores(*seq[i + la])
                for hp in range(2):
                    recip = scrp.tile([P, SC], F32, tag="recip", name="recip")
                    nc.vector.reciprocal_approx_fast(recip[:], d_ps[hp][:])
                    nc.vector.tensor_mul(
                        outT_qc[:, hs[hp], :], o_ps[hp][:], recip[:]
                    )

            def make_wo_blocks(qc, outT_qc):
                """16 [128,512] wo-projection blocks for query chunk qc:
                4 accumulating matmuls (one per head), a PSUM->SBUF copy
                alternating DVE/ACT, and the y output DMA."""
                work = []
                for sti in range(NSUB):
                    st = qc * NSUB + sti
                    stsl = slice(sti * P, (sti + 1) * P)
                    row = {}
                    for dc in range(D // SC):
                        dsl = slice(dc * SC, (dc + 1) * SC)
                        bi = len(work)

                        def blk(st=st, stsl=stsl, dsl=dsl, bi=bi, dc=dc, row=row):
                            y_ps = y2.tile([P, SC], F32, tag="y", name="y_ps")
                            for h in range(H_LOC):
                                nc.tensor.matmul(
                                    y_ps[:],
                                    outT_qc[:, h, stsl],
                                    wo_sb[:, h, dsl],
                                    start=(h == 0),
                                    stop=(h == H_LOC - 1),
                                )
                            if dc == 0:
                                row["ysb"] = yop.tile(
                                    [P, D], BF16, tag="ysb", name="y_sb"
                                )
                            y_sb = row["ysb"]
                            nc.vector.tensor_copy(y_sb[:, dsl], y_ps[:])
                            if dc == D // SC - 1:
                                eng = nc.sync if st % 2 == 0 else nc.gpsimd
                                eng.dma_start(
                                    y[st * P : (st + 1) * P, :], y_sb[:]
                                )

                        work.append(blk)
                return work

            pending = deque()
            if causal:
                xt_next = xt0
                for sc in range(NQC):
                    xt = xt_next
                    if sc + 1 < NQC:
                        xt_next = load_chunk(sc + 1)
                    qT_c = qpool.tile(
                        [P, H_LOC, SC], BF16, tag="qT", name=f"qT{sc}"
                    )
                    project_chunk(sc, xt, qT_c)
                    outT_qc = opool.tile(
                        [P, H_LOC, SC], BF16, tag="outT", name=f"outT{sc}"
                    )
                    attend_half(sc, 0, qT_c, outT_qc, pending)
                    attend_half(sc, 1, qT_c, outT_qc, pending)
                    pending.extend(make_wo_blocks(sc, outT_qc))
            else:
                xt_next = xt0
                for sc in range(NQC):
                    xt = xt_next
                    if sc + 1 < NQC:
                        xt_next = load_chunk(sc + 1)
                    project_chunk(sc, xt, qT_full)
                for qc in range(NQC):
                    outT_qc = opool.tile(
                        [P, H_LOC, SC], BF16, tag="outT", name=f"outT{qc}"
                    )
                    attend_half(qc, 0, qT_full, outT_qc, pending)
                    attend_half(qc, 1, qT_full, outT_qc, pending)
                    pending.extend(make_wo_blocks(qc, outT_qc))
            while pending:
                pending.popleft()()

    nc.compile()
    return nc


_NC_CACHE = {}


def _get_nc(causal: bool):
    if causal not in _NC_CACHE:
        _NC_CACHE[causal] = _build_core_kernel(causal)
    return _NC_CACHE[causal]


def _rope_perm_T() -> np.ndarray:
    # rotate_half as a matrix: (P_rh @ q)[d] = -q[d+HD/2] for d < HD/2,
    # q[d-HD/2] otherwise.  Returns P_rh.T for use as matmul lhsT.
    P_rh = np.zeros((HD, HD), dtype=np.float32)
    half = HD // 2
    for i in range(half):
        P_rh[i, half + i] = -1.0
        P_rh[half + i, i] = 1.0
    return np.ascontiguousarray(P_rh.T)


def _is_causal(m: np.ndarray) -> bool:
    tril = np.tril(np.ones((S, S), dtype=bool))
    if not np.all(m[tril] == 0.0):
        return False
    upper = m[~tril]
    return bool(upper.size == 0 or np.all(upper <= -1.0e8))


def _bf16(a: np.ndarray) -> np.ndarray:
    return np.ascontiguousarray(a, dtype=np.float32).astype(BF_NP)


# module-level: results of the last traced run (for test harnesses)
last_exec_time_ns = None
last_profile_json = None


def kernel(x, cos, sin, mask, wq, wk, wv, wo, _trace=False):
    x = np.asarray(x, dtype=np.float32)
    cos = np.asarray(cos, dtype=np.float32)
    sin = np.asarray(sin, dtype=np.float32)
    mask = np.asarray(mask, dtype=np.float32)
    wq = np.asarray(wq, dtype=np.float32)
    wk = np.asarray(wk, dtype=np.float32)
    wv = np.asarray(wv, dtype=np.float32)
    wo = np.asarray(wo, dtype=np.float32)

    m2d = mask.reshape(S, S)
    causal = _is_causal(m2d)
    nc = _get_nc(causal)

    scale = np.float32(np.sqrt(HD))
    cosT = _bf16(cos.T)
    sinT = _bf16(sin.T)
    ptT = _bf16(_rope_perm_T())
    ones_m = np.ones((P, P), dtype=BF_NP)

    def swizzle(a, nblk):
        # [nblk*P, cols] -> [P, nblk, cols] (ki-major rows for 1-segment DMA)
        return np.ascontiguousarray(
            a.reshape(nblk, P, -1).transpose(1, 0, 2)
        )

    if causal:
        maskT = np.ascontiguousarray((m2d[:SC, :SC] * scale).T)
        maskP = _bf16(swizzle(maskT, NSUB))
    else:
        maskT = np.ascontiguousarray((m2d * scale).T).astype(np.float32)

    xT = [_bf16(x[b].T) for b in range(B)]

    in_maps = []
    for c in range(N_CORES):
        b = c // (N_CORES // B)
        hg = c % (N_CORES // B)
        rows = slice(hg * HW, (hg + 1) * HW)
        # q/k blocks: [8, P, KO, HD], block i=(2h+t); v: [P, KO, 4*HD]
        qk = []
        for hl in range(H_LOC):
            h = hg * H_LOC + hl
            qk.append(swizzle(wq[h * HD : (h + 1) * HD].T, KO))
            qk.append(swizzle(wk[h * HD : (h + 1) * HD].T, KO))
        wqkP = np.ascontiguousarray(
            np.stack(qk).transpose(1, 0, 2, 3)
        )  # [P, 8, KO, HD]
        vcols = np.concatenate(
            [
                wv[(hg * H_LOC + hl) * HD : (hg * H_LOC + hl + 1) * HD].T
                for hl in range(H_LOC)
            ],
            axis=1,
        )  # [D, 4*HD]
        wvP = swizzle(vcols, KO)  # [P, KO, 4*HD]
        woP = swizzle(np.ascontiguousarray(wo[:, rows].T), H_LOC)  # [P,H,D]
        im = {
            "xT": xT[b],
            "wqkP": _bf16(wqkP),
            "wvP": _bf16(wvP),
            "woP": _bf16(woP),
            "cosT": cosT,
            "sinT": sinT,
            "PT": ptT,
            "ones": ones_m,
        }
        if causal:
            im["maskP"] = maskP
        else:
            im["maskT"] = maskT
        in_maps.append(im)

    kw = {}
    if _trace:
        kw = dict(trace=True)
    res = run_bass_kernel_spmd(
        nc, in_maps, core_ids=list(range(N_CORES)), **kw
    )
    global last_exec_time_ns, last_profile_json
    last_exec_time_ns = res.exec_time_ns
    last_profile_json = res.profile_json

    out = np.empty((B, S, D), dtype=np.float32)
    gs = N_CORES // B
    for b in range(B):
        acc = res.results[b * gs]["y"].astype(np.float32)
        for g in range(1, gs):
            acc += res.results[b * gs + g]["y"].astype(np.float32)
        out[b] = acc
    return out


# revision 39
# speedup vs baseline: 1.1709x; 1.1709x over previous
"""Trainium2 8-core kernel for nn_Attention_27530740367526.

Multi-head causal attention (B=2, S=2048, D=2048, H=16, HD=128) with RoPE,
sharded batch x head-group across 8 NeuronCores: core c handles batch c//4
and heads [4*(c%4), 4*(c%4)+4).  Each core computes q/k/v projections
(+RoPE), attention for its 4 heads, and its heads' slice of the wo
projection -- a partial [S, D] output.  The host sums the 4 partials per
batch (the row-parallel wo "all-reduce" is a host-side unshard).

All matmul operands are bf16 (PSUM accumulation is fp32), which runs at
full PE rate, halves DMA/SBUF traffic vs f32r, and keeps LDWEIGHTS cheap.
Everything lives in "transposed land": qT/kT are [head_dim, seq] with
head-dim on partitions, so scores come out transposed ([k, q]), the
softmax denominator is an all-ones-column matmul (partition-broadcast
denominator for free), and PV / wo consume natural layouts with zero
on-device transposes.  RoPE's rotate-half is a 128x128 permutation matmul.

Schedule per core (single pass over all 4 heads -- y is written once):
  P0 A0 P1 A1+W0 P2 A2+W1 P3 A3+W2 W3
where P(sc) projects q/k/v for 512-seq chunk sc (dense PE phase, next x
chunk prefetched via split DMA queues), A(qc) runs causal attention for
query chunk qc as two 2-head interleaved softmax chains, and W(qc) is the
wo projection of chunk qc cut into 16 [128,512] blocks used as PE filler
inside the NEXT attention phase's exp-wait bubbles (one 4-matmul block
between a step's exp and its PV keeps the PE continuously busy, which
also keeps the PE p-state clock at max).

Further scheduling details that the trace showed matter:
- diagonal k-blocks are column-trimmed (scores/exp/PV/denominator only
  touch q >= j*128; the mask add is a single [128,128] bf16 triangle);
- every DMA is one contiguous segment per partition row (inputs are
  pre-swizzled on the host) and the initial weight stream is split
  across the scalar/sync/gpsimd queues in chain-consumption order so
  the first projection phase runs at DMA pace from ~4 us;
- PSUM: 4 banks rotate o/d accumulators and projection chains, 2 banks
  pipeline scores (lookahead 2; chunk 0 borrows the idle wo banks for
  lookahead 4), 2 banks ping-pong wo blocks;
- PSUM->SBUF copies and RoPE elementwise run on DVE, exp on ACT, and
  DMA issue on sync/gpsimd, keeping every co-engine under ~60% so the
  PE's dependency chains never back up.
"""

import sys

if "/opt/trn_rl_repo" not in sys.path:
    sys.path.insert(0, "/opt/trn_rl_repo")

from collections import deque

import ml_dtypes
import numpy as np

import concourse.bacc as bacc
import concourse.mybir as mybir
import concourse.tile as tile
from concourse.bass_utils import run_bass_kernel_spmd

F32 = mybir.dt.float32
BF16 = mybir.dt.bfloat16
AF = mybir.ActivationFunctionType
BF_NP = ml_dtypes.bfloat16

N_HEADS = 16
N_CORES = 8
B, S, D = 2, 2048, 2048
HD = D // N_HEADS
H_LOC = N_HEADS // (N_CORES // B)  # 4 heads per core
HW = H_LOC * HD                    # 512 wo rows per core
SC = 512                           # seq chunk (matmul moving free dim)
P = 128
KO = D // P                        # 16 contraction subtiles
NQC = S // SC                      # 4 q-chunks
NSUB = SC // P                     # 4 128-blocks per chunk
NST = S // P                       # 16 s-tiles
LA = 2                             # scores-tile software pipeline depth


def _build_core_kernel(causal: bool):
    inv_sqrt_hd = 1.0 / float(np.sqrt(HD))

    nc = bacc.Bacc(None, target_bir_lowering=False)

    # All inputs are pre-swizzled on the host so every DMA descriptor is
    # one segment per partition row (contiguous 1-16 KB rows): fat issues
    # were measured at 3-12 us on the issuing engine otherwise.
    xT = nc.dram_tensor("xT", [D, S], BF16, kind="ExternalInput")
    wqkP = nc.dram_tensor("wqkP", [P, 8, KO, HD], BF16, kind="ExternalInput")
    wvP = nc.dram_tensor("wvP", [P, KO, 4 * HD], BF16, kind="ExternalInput")
    woP = nc.dram_tensor("woP", [P, H_LOC, D], BF16, kind="ExternalInput")
    cosT = nc.dram_tensor("cosT", [HD, S], BF16, kind="ExternalInput")
    sinT = nc.dram_tensor("sinT", [HD, S], BF16, kind="ExternalInput")
    PT = nc.dram_tensor("PT", [HD, HD], BF16, kind="ExternalInput")
    ones = nc.dram_tensor("ones", [P, P], BF16, kind="ExternalInput")
    if causal:
        # bf16 is plenty: mask entries are 0 or ~-1e10, and exp of any
        # value <= -1e8 is 0 either way
        maskP = nc.dram_tensor("maskP", [P, NSUB, SC], BF16, kind="ExternalInput")
    else:
        maskT = nc.dram_tensor("maskT", [S, S], F32, kind="ExternalInput")
    y = nc.dram_tensor("y", [S, D], BF16, kind="ExternalOutput")

    xT_r = xT.rearrange("(ko ki) s -> ki ko s", ki=P)

    with tile.TileContext(nc) as tc:
        with (
            tc.tile_pool(name="persist", bufs=1) as persist,
            tc.tile_pool(name="xa", bufs=2) as xa,
            tc.tile_pool(name="qp", bufs=2) as qpool,
            tc.tile_pool(name="op", bufs=2) as opool,
            tc.tile_pool(name="plainp", bufs=3) as plainp,
            tc.tile_pool(name="ropet", bufs=2) as ropet,
            tc.tile_pool(name="ep", bufs=5) as ep,
            tc.tile_pool(name="yo", bufs=3) as yop,
            tc.tile_pool(name="scr", bufs=2) as scrp,
            tc.tile_pool(name="gm", bufs=3) as gmp,
            tc.tile_pool(name="acc", bufs=4, space="PSUM") as accp,
            tc.tile_pool(name="sc2", bufs=LA, space="PSUM") as sc2,
            tc.tile_pool(name="y2", bufs=2, space="PSUM") as y2,
        ):
            # ---- initial DMAs.  All weight blocks go on the scalar queue
            # (cheap single-segment issues; the scalar engine runs nothing
            # else early since PSUM->SBUF copies live on DVE); x chunk 0
            # round-robins sync/gpsimd per ko so the first chains can sweep
            # as subtiles land; cos/sin/mask/wo follow behind.
            wqk_sb = persist.tile([P, 8, KO, HD], BF16, tag="w", name="wqk_sb")
            wv_sb = persist.tile([P, KO, 4 * HD], BF16, tag="wv", name="wv_sb")
            xt0 = xa.tile([P, KO, SC], BF16, tag="xt", name="xt0")

            KH = KO // 2

            def w_half(eng, i, hf):
                eng.dma_start(
                    wqk_sb[:, i, hf * KH : (hf + 1) * KH],
                    wqkP[:, i, hf * KH : (hf + 1) * KH],
                )

            def wv_quarter(eng, q):
                eng.dma_start(
                    wv_sb[:, q * 4 : (q + 1) * 4], wvP[:, q * 4 : (q + 1) * 4]
                )

            # scalar: chains 0-3 weights (+late v quarters); sync carries
            # x chunk 0 evens + small persists, then chains 4-7 weights
            for i in range(4):
                w_half(nc.scalar, i, 0)
                w_half(nc.scalar, i, 1)
            for ko in range(KO):
                eng = nc.sync if ko % 2 == 0 else nc.gpsimd
                eng.dma_start(xt0[:, ko], xT_r[:, ko, 0:SC])
            wv_quarter(nc.scalar, 2)
            wv_quarter(nc.scalar, 3)
            cos_sb = persist.tile([P, S], BF16, tag="cos", name="cos_sb")
            nc.sync.dma_start(cos_sb[:], cosT[:])
            sin_sb = persist.tile([P, S], BF16, tag="sin", name="sin_sb")
            nc.gpsimd.dma_start(sin_sb[:], sinT[:])
            pt_sb = persist.tile([P, HD], BF16, tag="pt", name="pt_sb")
            nc.sync.dma_start(pt_sb[:], PT[:])
            ones_sb = persist.tile([P, P], BF16, tag="ones", name="ones_sb")
            nc.sync.dma_start(ones_sb[:], ones[:])
            for i in range(4, 8):
                w_half(nc.gpsimd, i, 0)
                w_half(nc.gpsimd, i, 1)
            wv_quarter(nc.sync, 0)
            wv_quarter(nc.sync, 1)
            if causal:
                mask_sb = persist.tile([P, NSUB, SC], BF16, tag="mask", name="mask_sb")
                nc.scalar.dma_start(mask_sb[:], maskP[:])
            wo_sb = persist.tile([P, H_LOC, D], BF16, tag="wo", name="wo_sb")
            nc.gpsimd.dma_start(wo_sb[:], woP[:])

            kT_sb = persist.tile([P, H_LOC, S], BF16, tag="kT", name="kT_sb")
            v_sb = persist.tile([P, NST, H_LOC * HD], BF16, tag="v", name="v_sb")
            qT_full = (
                persist.tile([P, H_LOC, S], BF16, tag="qTf", name="qT_full")
                if not causal
                else None
            )

            def load_chunk(sc):
                # prefetched a full phase ahead -> two half-descriptors
                ssl = slice(sc * SC, (sc + 1) * SC)
                xt = xa.tile([P, KO, SC], BF16, tag="xt", name=f"xt{sc}")
                nc.sync.dma_start(xt[:, : KO // 2], xT_r[:, : KO // 2, ssl])
                nc.gpsimd.dma_start(xt[:, KO // 2 :], xT_r[:, KO // 2 :, ssl])
                return xt

            def project_chunk(sc, xt, qT_c, do_v=True):
                """q/k (+RoPE) and v projections for seq chunk sc.  The
                RoPE for chain i is emitted during chain i+1's matmuls so
                the rotate-half matmul never stalls the PE on the
                PSUM->SBUF copy."""
                ssl = slice(sc * SC, (sc + 1) * SC)
                pending_rope = []

                def flush_rope():
                    for h, t, plain, dst in pending_rope:
                        rot = sc2.tile([P, SC], F32, tag="sc", name="rot")
                        nc.tensor.matmul(rot[:], pt_sb[:], plain[:])
                        pc = ropet.tile([P, SC], F32, tag="pc", name="pc")
                        nc.vector.tensor_mul(pc[:], plain[:], cos_sb[:, ssl])
                        t2 = ropet.tile([P, SC], F32, tag="t2", name="t2")
                        nc.vector.tensor_mul(t2[:], rot[:], sin_sb[:, ssl])
                        nc.vector.tensor_add(dst, pc[:], t2[:])
                    pending_rope.clear()

                for h in range(H_LOC):
                    for t in range(2):  # 0=q, 1=k
                        ps = accp.tile([P, SC], F32, tag="acc", name="ps")
                        for ko in range(KO):
                            nc.tensor.matmul(
                                ps[:],
                                wqk_sb[:, 2 * h + t, ko],
                                xt[:, ko],
                                start=(ko == 0),
                                stop=(ko == KO - 1),
                            )
                        plain = plainp.tile([P, SC], BF16, tag="plain", name="plain")
                        nc.vector.tensor_copy(plain[:], ps[:])
                        if t == 0:
                            dst = qT_c[:, h, ssl] if qT_c is qT_full else qT_c[:, h, :]
                        else:
                            dst = kT_sb[:, h, ssl]
                        flush_rope()
                        pending_rope.append((h, t, plain, dst))

                for sti in range(NSUB):
                    if do_v:
                        v_chain(sc, xt, sti, accp)
                    flush_rope()
                if not do_v:
                    flush_rope()

            def v_chain(sc, xt, sti, pool):
                st = sc * NSUB + sti
                lsl = slice(sti * P, (sti + 1) * P)
                psv = pool.tile(
                    [P, H_LOC * HD], F32,
                    tag="acc" if pool is accp else "y", name="psv",
                )
                for ko in range(KO):
                    nc.tensor.matmul(
                        psv[:],
                        xt[:, ko, lsl],
                        wv_sb[:, ko],
                        start=(ko == 0),
                        stop=(ko == KO - 1),
                    )
                nc.vector.tensor_copy(v_sb[:, st, :], psv[:])

            def attend_half(qc, half, qT_c, outT_qc, fillers):
                """Attention for query chunk qc, heads (2*half, 2*half+1)
                interleaved per k-block.  One filler block (4 wo matmuls)
                is drained between a step's exp and its PV matmul so the
                PE bridges the exp latency with independent work.

                Diagonal k-blocks (j = kb - qc*NSUB >= 0) are column-
                trimmed: only q columns >= j*P can attend to that block,
                so scores/exp/PV/denominator run on [:, j*P:] and the mask
                add touches just the [128,128] triangle."""
                nkb = (qc + 1) * NSUB if causal else NST
                hs = (2 * half, 2 * half + 1)
                qt = {}
                o_ps = {}
                d_ps = {}
                for hp in range(2):
                    qt[hp] = (
                        qT_c[:, hs[hp], qc * SC : (qc + 1) * SC]
                        if qT_c is qT_full
                        else qT_c[:, hs[hp], :]
                    )
                    o_ps[hp] = accp.tile([P, SC], F32, tag="acc", name=f"o{hp}")
                    d_ps[hp] = accp.tile([P, SC], F32, tag="acc", name=f"d{hp}")
                stile = {}

                def cotrim(kb):
                    j = kb - qc * NSUB
                    return P * j if (causal and j > 0) else 0

                # qc 0 has no wo fillers; deepen its scores lookahead by
                # borrowing the (idle until A(1)) y2 PSUM slots
                la = 4 if (causal and qc == 0) else LA
                scnt = [0]

                def emit_scores(kb, hp):
                    co = cotrim(kb)
                    if la == 4 and scnt[0] % 2 == 1:
                        t_ = y2.tile([P, SC], F32, tag="y", name="sc_y")
                    else:
                        t_ = sc2.tile([P, SC], F32, tag="sc", name="scores")
                    scnt[0] += 1
                    nc.tensor.matmul(
                        t_[:, co:],
                        kT_sb[:, hs[hp], kb * P : (kb + 1) * P],
                        qt[hp][:, co:],
                        skip_group_check=True,
                    )
                    if causal:
                        j = kb - qc * NSUB
                        if j >= 0:
                            nc.vector.tensor_add(
                                t_[:, co : co + P],
                                t_[:, co : co + P],
                                mask_sb[:, j, co : co + P],
                            )
                    else:
                        if hp == 0:
                            mt = gmp.tile([P, SC], F32, tag="mt", name="mt")
                            nc.sync.dma_start(
                                mt[:],
                                maskT[
                                    kb * P : (kb + 1) * P,
                                    qc * SC : (qc + 1) * SC,
                                ],
                            )
                            stile[("m", kb)] = mt
                        nc.vector.tensor_add(t_[:], t_[:], stile[("m", kb)][:])
                    stile[(kb, hp)] = t_

                seq = [(kb, hp) for kb in range(nkb) for hp in range(2)]
                for s_ in seq[:la]:
                    emit_scores(*s_)
                for i, (kb, hp) in enumerate(seq):
                    co = cotrim(kb)
                    e = ep.tile([P, SC], BF16, tag="e", name="e")
                    nc.scalar.activation(
                        e[:, co:],
                        stile.pop((kb, hp))[:, co:],
                        AF.Exp,
                        scale=inv_sqrt_hd,
                    )
                    if fillers and i % 2 == 0:
                        fillers.popleft()()
                    h = hs[hp]
                    nc.tensor.matmul(
                        o_ps[hp][:, co:],
                        v_sb[:, kb, h * HD : (h + 1) * HD],
                        e[:, co:],
                        start=(kb == 0),
                        stop=(kb == nkb - 1),
                        skip_group_check=True,
                    )
                    nc.tensor.matmul(
                        d_ps[hp][:, co:],
                        ones_sb[:],
                        e[:, co:],
                        start=(kb == 0),
                        stop=(kb == nkb - 1),
                        skip_group_check=True,
                    )
                    if i + la < len(seq):
                        emit_scores(*seq[i + la])
                for hp in range(2):
                    recip = scrp.tile([P, SC], F32, tag="recip", name="recip")
                    nc.vector.reciprocal_approx_fast(recip[:], d_ps[hp][:])
                    nc.vector.tensor_mul(
                        outT_qc[:, hs[hp], :], o_ps[hp][:], recip[:]
                    )

            def make_wo_blocks(qc, outT_qc):
                """16 [128,512] wo-projection blocks for query chunk qc:
                4 accumulating matmuls (one per head), a PSUM->SBUF copy
                alternating DVE/ACT, and the y output DMA."""
                work = []
                for sti in range(NSUB):
                    st = qc * NSUB + sti
                    stsl = slice(sti * P, (sti + 1) * P)
                    row = {}
                    for dc in range(D // SC):
                        dsl = slice(dc * SC, (dc + 1) * SC)
                        bi = len(work)

                        def blk(st=st, stsl=stsl, dsl=dsl, bi=bi, dc=dc, row=row):
                            y_ps = y2.tile([P, SC], F32, tag="y", name="y_ps")
                            for h in range(H_LOC):
                                nc.tensor.matmul(
                                    y_ps[:],
                                    outT_qc[:, h, stsl],
                                    wo_sb[:, h, dsl],
                                    start=(h == 0),
                                    stop=(h == H_LOC - 1),
                                )
                            if dc == 0:
                                row["ysb"] = yop.tile(
                                    [P, D], BF16, tag="ysb", name="y_sb"
                                )
                            y_sb = row["ysb"]
                            nc.vector.tensor_copy(y_sb[:, dsl], y_ps[:])
                            if dc == D // SC - 1:
                                eng = nc.sync if st % 2 == 0 else nc.gpsimd
                                eng.dma_start(
                                    y[st * P : (st + 1) * P, :], y_sb[:]
                                )

                        work.append(blk)
                return work

            pending = deque()
            if causal:
                xt_next = xt0
                for sc in range(NQC):
                    xt = xt_next
                    if sc + 1 < NQC:
                        xt_next = load_chunk(sc + 1)
                    qT_c = qpool.tile(
                        [P, H_LOC, SC], BF16, tag="qT", name=f"qT{sc}"
                    )
                    project_chunk(sc, xt, qT_c)
                    outT_qc = opool.tile(
                        [P, H_LOC, SC], BF16, tag="outT", name=f"outT{sc}"
                    )
                    attend_half(sc, 0, qT_c, outT_qc, pending)
                    attend_half(sc, 1, qT_c, outT_qc, pending)
                    pending.extend(make_wo_blocks(sc, outT_qc))
            else:
                xt_next = xt0
                for sc in range(NQC):
                    xt = xt_next
                    if sc + 1 < NQC:
                        xt_next = load_chunk(sc + 1)
                    project_chunk(sc, xt, qT_full)
                for qc in range(NQC):
                    outT_qc = opool.tile(
                        [P, H_LOC, SC], BF16, tag="outT", name=f"outT{qc}"
                    )
                    attend_half(qc, 0, qT_full, outT_qc, pending)
                    attend_half(qc, 1, qT_full, outT_qc, pending)
                    pending.extend(make_wo_blocks(qc, outT_qc))
            while pending:
                pending.popleft()()

    nc.compile()
    return nc


_NC_CACHE = {}


def _get_nc(causal: bool):
    if causal not in _NC_CACHE:
        _NC_CACHE[causal] = _build_core_kernel(causal)
    return _NC_CACHE[causal]


def _rope_perm_T() -> np.ndarray:
    # rotate_half as a matrix: (P_rh @ q)[d] = -q[d+HD/2] for d < HD/2,
    # q[d-HD/2] otherwise.  Returns P_rh.T for use as matmul lhsT.
    P_rh = np.zeros((HD, HD), dtype=np.float32)
    half = HD // 2
    for i in range(half):
        P_rh[i, half + i] = -1.0
        P_rh[half + i, i] = 1.0
    return np.ascontiguousarray(P_rh.T)


def _is_causal(m: np.ndarray) -> bool:
    tril = np.tril(np.ones((S, S), dtype=bool))
    if not np.all(m[tril] == 0.0):
        return False
    upper = m[~tril]
    return bool(upper.size == 0 or np.all(upper <= -1.0e8))


def _bf16(a: np.ndarray) -> np.ndarray:
    return np.ascontiguousarray(a, dtype=np.float32).astype(BF_NP)


# module-level: results of the last traced run (for test harnesses)
last_exec_time_ns = None
last_profile_json = None


def kernel(x, cos, sin, mask, wq, wk, wv, wo, _trace=False):
    x = np.asarray(x, dtype=np.float32)
    cos = np.asarray(cos, dtype=np.float32)
    sin = np.asarray(sin, dtype=np.float32)
    mask = np.asarray(mask, dtype=np.float32)
    wq = np.asarray(wq, dtype=np.float32)
    wk = np.asarray(wk, dtype=np.float32)
    wv = np.asarray(wv, dtype=np.float32)
    wo = np.asarray(wo, dtype=np.float32)

    m2d = mask.reshape(S, S)
    causal = _is_causal(m2d)
    nc = _get_nc(causal)

    scale = np.float32(np.sqrt(HD))
    cosT = _bf16(cos.T)
    sinT = _bf16(sin.T)
    ptT = _bf16(_rope_perm_T())
    ones_m = np.ones((P, P), dtype=BF_NP)

    def swizzle(a, nblk):
        # [nblk*P, cols] -> [P, nblk, cols] (ki-major rows for 1-segment DMA)
        return np.ascontiguousarray(
            a.reshape(nblk, P, -1).transpose(1, 0, 2)
        )

    if causal:
        maskT = np.ascontiguousarray((m2d[:SC, :SC] * scale).T)
        maskP = _bf16(swizzle(maskT, NSUB))
    else:
        maskT = np.ascontiguousarray((m2d * scale).T).astype(np.float32)

    xT = [_bf16(x[b].T) for b in range(B)]

    in_maps = []
    for c in range(N_CORES):
        b = c // (N_CORES // B)
        hg = c % (N_CORES // B)
        rows = slice(hg * HW, (hg + 1) * HW)
        # q/k blocks: [8, P, KO, HD], block i=(2h+t); v: [P, KO, 4*HD]
        qk = []
        for hl in range(H_LOC):
            h = hg * H_LOC + hl
            qk.append(swizzle(wq[h * HD : (h + 1) * HD].T, KO))
            qk.append(swizzle(wk[h * HD : (h + 1) * HD].T, KO))
        wqkP = np.ascontiguousarray(
            np.stack(qk).transpose(1, 0, 2, 3)
        )  # [P, 8, KO, HD]
        vcols = np.concatenate(
            [
                wv[(hg * H_LOC + hl) * HD : (hg * H_LOC + hl + 1) * HD].T
                for hl in range(H_LOC)
            ],
            axis=1,
        )  # [D, 4*HD]
        wvP = swizzle(vcols, KO)  # [P, KO, 4*HD]
        woP = swizzle(np.ascontiguousarray(wo[:, rows].T), H_LOC)  # [P,H,D]
        im = {
            "xT": xT[b],
            "wqkP": _bf16(wqkP),
            "wvP": _bf16(wvP),
            "woP": _bf16(woP),
            "cosT": cosT,
            "sinT": sinT,
            "PT": ptT,
            "ones": ones_m,
        }
        if causal:
            im["maskP"] = maskP
        else:
            im["maskT"] = maskT
        in_maps.append(im)

    kw = {}
    if _trace:
        kw = dict(trace=True)
    res = run_bass_kernel_spmd(
        nc, in_maps, core_ids=list(range(N_CORES)), **kw
    )
    global last_exec_time_ns, last_profile_json
    last_exec_time_ns = res.exec_time_ns
    last_profile_json = res.profile_json

    out = np.empty((B, S, D), dtype=np.float32)
    gs = N_CORES // B
    for b in range(B):
        acc = res.results[b * gs]["y"].astype(np.float32)
        for g in range(1, gs):
            acc += res.results[b * gs + g]["y"].astype(np.float32)
        out[b] = acc
    return out
